# revision 35
# baseline (speedup 1.0000x reference)
"""CRF-RNN layer (nn_CRF_RNN_Layer) as a Bass/Tile kernel on 8 trn2 NeuronCores.

Math (reference):
    N = 96*96 pixels, C = 21 classes, 5 mean-field iterations.
    k_spatial / k_bilateral are [N, N] Gaussian kernels; per iteration:
        p = softmax(q); S = Ks @ p; Bi = Kb @ p
        pairwise = (S * ws + Bi * wb) @ C.T;  q = u - pairwise

Device strategy:
    - Row-shard outputs over 8 cores (1152 rows each).
    - Channels padded 21 -> 32; pad logits are -1e30 so softmax pads are 0.
    - Per-channel weights + compat fold on host: CsF[c,k] = ws[c]*C[k,c].
    - Big matmuls in out.T form: S.T[c, i] = sum_j p[j, c] * K[j, i],
      with lhsT = p tiles [128, 32] bf16 and rhs = K tiles [128, 1152] bf16.
    - Kb (data-dependent) is built on device: X = Faug_j . Gaug_i via a
      7-contraction matmul, exp on the scalar engine, stored bf16 (mostly
      SBUF-resident; every 4th tile spilled to HBM and re-streamed).
    - Ks depends only on pixel positions -> host constant, streamed bf16.
    - Each core softmaxes its own band and the cores AllGather the bf16
      probabilities (74KB/rank) in a partition-major block layout so all
      DMA runs are contiguous.
"""

import numpy as np
import ml_dtypes

from concourse import bacc, mybir, tile
from concourse.bass_utils import run_bass_kernel_spmd

H, W, C = 96, 96, 21
THETA_ALPHA, THETA_BETA, THETA_GAMMA = 8.0, 0.125, 3.0
NITER = 5
NCORES = 8
N = H * W                     # 9216
BAND = N // NCORES            # 1152 rows per core
CP = 32                       # padded channels
TJ = N // 128                 # 72 j-tiles
TB = BAND // 128              # 9 band tiles
SPILL_MOD = 4                 # j-tiles with jt % 4 == 2 are spilled to HBM
GRP = 6                       # gpos/ks j-tiles per load group
SGRP = 5                 # spilled-Kb j-tiles per streaming DMA
NEG = -1.0e30
CHUNKS = [(0, 512), (512, 512), (1024, 128)]   # psum-bank-aligned N splits of 1152

SPILLED = []
S_IDX = {jt: s for s, jt in enumerate(SPILLED)}
NSPILL = len(SPILLED)
RES_IDX = {}
for jt in range(TJ):
    if jt not in S_IDX:
        RES_IDX[jt] = len(RES_IDX)
NRES = len(RES_IDX)

_CACHE = {}


def _build_nc():
    nc = bacc.Bacc("TRN2", target_bir_lowering=False, debug=False, num_devices=NCORES)
    f32 = mybir.dt.float32
    bf16 = mybir.dt.bfloat16

    f16 = mybir.dt.float16
    at_d = nc.declare_dram_parameter("atc", [5, N], f16, isOutput=False)
    bt_d = nc.declare_dram_parameter("btc", [5, BAND], f16, isOutput=False)
    gpos_d = nc.declare_dram_parameter("gpos", [N, BAND], mybir.dt.float8e4, isOutput=False)
    fp8 = mybir.dt.float8e4
    kst_d = nc.declare_dram_parameter("kst", [N, BAND], fp8, isOutput=False)
    uband_d = nc.declare_dram_parameter("uband", [128, TB * CP], f32, isOutput=False)
    csf_d = nc.declare_dram_parameter("csf", [CP, CP], f32, isOutput=False)
    cbf_d = nc.declare_dram_parameter("cbf", [CP, CP], f32, isOutput=False)
    out_d = nc.declare_dram_parameter("out", [128, TB * CP], f32, isOutput=True)

    with tile.TileContext(nc) as tc:
        with (
            tc.tile_pool(name="kres", bufs=1) as kres_pool,
            tc.tile_pool(name="state", bufs=1) as state,
            tc.tile_pool(name="small", bufs=1) as small,
            tc.tile_pool(name="dram", bufs=1, space="DRAM") as dram,
        ):
            # ---- constants ----
            csf = state.tile([CP, CP], f32, tag="csf")
            cbf = state.tile([CP, CP], f32, tag="cbf")
            u_band = state.tile([128, TB * CP], f32, tag="uband")
            bt = state.tile([5, BAND], f16, tag="bt")
            nc.sync.dma_start(bt[:], bt_d[:])
            nc.scalar.dma_start(csf[:], csf_d[:])
            nc.scalar.dma_start(cbf[:], cbf_d[:])
            nc.scalar.dma_start(u_band[:], uband_d[:])

            kb_res = kres_pool.tile([128, TJ * BAND], fp8, tag="kbres")
            ks_res = kres_pool.tile([128, TJ * BAND], fp8, tag="ksres")

            # ---- iteration-0: band softmax of u + AllGather (overlaps build) ----
            def band_softmax_ag(src_tile, it):
                eb = small.tile([128, TB * CP], f32, tag="eb")
                nc.scalar.activation(
                    eb[:], src_tile[:], mybir.ActivationFunctionType.Exp
                )
                sb = small.tile([128, TB], f32, tag="sb")
                nc.vector.tensor_reduce(
                    sb[:],
                    eb.rearrange("p (t c) -> p t c", c=CP)[:],
                    axis=mybir.AxisListType.X,
                    op=mybir.AluOpType.add,
                )
                rb = small.tile([128, TB], f32, tag="rb")
                nc.vector.reciprocal(rb[:], sb[:])
                pband = small.tile([128, TB * CP], fp8, tag="pband")
                nc.vector.tensor_tensor(
                    pband.rearrange("p (t c) -> p t c", c=CP)[:],
                    eb.rearrange("p (t c) -> p t c", c=CP)[:],
                    rb[:].unsqueeze(2).to_broadcast((128, TB, CP)),
                    op=mybir.AluOpType.mult,
                )
                ag_in = dram.tile([128 * TB * CP], fp8, tag=f"agin{it}")
                ag_out = dram.tile(
                    [NCORES * 128 * TB * CP], fp8,
                    addr_space="Shared", tag=f"agout{it}",
                )
                nc.gpsimd.dma_start(
                    ag_in.rearrange("(p f) -> p f", p=128)[:], pband[:]
                )
                nc.gpsimd.collective_compute(
                    "AllGather",
                    mybir.AluOpType.bypass,
                    ins=[ag_in[:]],
                    outs=[ag_out[:]],
                    replica_groups=[list(range(NCORES))],
                )
                return ag_out

            ag_out = band_softmax_ag(u_band, 0)

            # ---- band halves A = ic 0..3, B = ic 4..8: the AG of half A
            #      overlaps the B-half matmuls ----
            HALVES = [  # (col_off, col_len, n_ictiles, psum chunk splits)
                (0, 512, 4, [(0, 512)]),
                (512, 640, 5, [(0, 512), (512, 128)]),
            ]

            def half_tail(it, h, acc_s, acc_b):
                """pairwise + qnew + softmax + AG for one half-band."""
                off, ln, nt, _ = HALVES[h]
                st_sb = state.tile([CP, ln], f32, tag=f"stsb{h}")
                bit_sb = state.tile([CP, ln], f32, tag=f"bitsb{h}")
                nc.scalar.copy(st_sb[:], acc_s[:])
                nc.vector.tensor_copy(bit_sb[:], acc_b[:])
                pw = pw_pool.tile([128, nt * CP], f32, tag=f"pw{h}")
                for ic in range(nt):
                    nc.tensor.matmul(
                        pw[:, ic * CP : (ic + 1) * CP],
                        st_sb[:, ic * 128 : (ic + 1) * 128],
                        csf[:], start=True, stop=False,
                    )
                    nc.tensor.matmul(
                        pw[:, ic * CP : (ic + 1) * CP],
                        bit_sb[:, ic * 128 : (ic + 1) * 128],
                        cbf[:], start=False, stop=True,
                    )
                qnew = small.tile([128, nt * CP], f32, tag=f"qnew{h}")
                nc.vector.tensor_tensor(
                    qnew[:], u_band[:, off // 4 : off // 4 + nt * CP], pw[:],
                    op=mybir.AluOpType.subtract,
                )
                if it == NITER - 1:
                    nc.sync.dma_start(
                        out_d[:, off // 4 : off // 4 + nt * CP], qnew[:]
                    )
                    return None
                eb = small.tile([128, nt * CP], f32, tag=f"eb{h}")
                nc.scalar.activation(
                    eb[:], qnew[:], mybir.ActivationFunctionType.Exp
                )
                sb = small.tile([128, nt], f32, tag=f"sb{h}")
                nc.vector.tensor_reduce(
                    sb[:], eb.rearrange("p (t c) -> p t c", c=CP)[:],
                    axis=mybir.AxisListType.X, op=mybir.AluOpType.add,
                )
                rb = small.tile([128, nt], f32, tag=f"rb{h}")
                nc.vector.reciprocal(rb[:], sb[:])
                pband = small.tile([128, nt * CP], fp8, tag=f"pband{h}")
                nc.vector.tensor_tensor(
                    pband.rearrange("p (t c) -> p t c", c=CP)[:],
                    eb.rearrange("p (t c) -> p t c", c=CP)[:],
                    rb[:].unsqueeze(2).to_broadcast((128, nt, CP)),
                    op=mybir.AluOpType.mult,
                )
                ag_in = dram.tile([128 * nt * CP], fp8, tag=f"agin{it + 1}{h}")
                ago = dram.tile(
                    [NCORES * 128 * nt * CP], fp8,
                    addr_space="Shared", tag=f"agout{it + 1}{h}",
                )
                nc.gpsimd.dma_start(
                    ag_in.rearrange("(p f) -> p f", p=128)[:], pband[:]
                )
                nc.gpsimd.collective_compute(
                    "AllGather", mybir.AluOpType.bypass,
                    ins=[ag_in[:]], outs=[ago[:]],
                    replica_groups=[list(range(NCORES))],
                )
                return ago

            if True:
                # ---- build Kb = Gpos * exp(color matmul) ----
                with (
                    tc.tile_pool(name="bpsum", bufs=2, space="PSUM") as bpsum,
                    tc.tile_pool(name="atpool", bufs=2) as atpool,
                    tc.tile_pool(name="gstage", bufs=3) as gstage,
                    tc.tile_pool(name="gpstream", bufs=2) as gp_pool,
                ):
                    at_chunk = None
                    gp_grp = None
                    for jt in range(TJ):
                        if jt % GRP == 0:
                            at_chunk = atpool.tile([5, GRP * 128], f16, tag="at")
                            nc.sync.dma_start(
                                at_chunk[:], at_d[:, jt * 128 : (jt + GRP) * 128]
                            )
                            gp_grp = gp_pool.tile([128, GRP * BAND], fp8, tag="gp")
                            nc.sync.dma_start(
                                gp_grp.rearrange("p (g i) -> p g i", g=GRP)[:],
                                gpos_d[jt * 128 : (jt + GRP) * 128, :].rearrange(
                                    "(g p) i -> p g i", p=128
                                ),
                            )
                            if (jt // GRP) % 2 == 0:
                                nc.scalar.dma_start(
                                    ks_res[
                                        :, jt * BAND : (jt + GRP) * BAND
                                    ].rearrange("p (g i) -> p g i", g=GRP)[:],
                                    kst_d[
                                        jt * 128 : (jt + GRP) * 128, :
                                    ].rearrange("(g p) i -> p g i", p=128),
                                )
                        lhs = at_chunk[:, (jt % GRP) * 128 : (jt % GRP) * 128 + 128]
                        xp = bpsum.tile([128, BAND], f32, tag="xp")
                        for off, ln in CHUNKS:
                            nc.tensor.matmul(
                                xp[:, off : off + ln], lhs, bt[:, off : off + ln],
                                start=True, stop=True,
                            )
                        gc = gstage.tile([128, BAND], bf16, tag="gc")
                        nc.scalar.activation(
                            gc[:], xp[:], mybir.ActivationFunctionType.Exp
                        )
                        gp = (jt % GRP) * BAND
                        dest = kb_res[:, jt * BAND : (jt + 1) * BAND]
                        nc.vector.tensor_tensor(
                            dest, gc[:], gp_grp[:, gp : gp + BAND],
                            op=mybir.AluOpType.mult,
                        )

                # deferred half of the resident Ks load (runs under iter 0)
                for jt in range(GRP, TJ, 2 * GRP):
                    nc.scalar.dma_start(
                        ks_res[:, jt * BAND : (jt + GRP) * BAND].rearrange(
                            "p (g i) -> p g i", g=GRP
                        )[:],
                        kst_d[jt * 128 : (jt + GRP) * 128, :].rearrange(
                            "(g p) i -> p g i", p=128
                        ),
                    )

                with (
                    tc.tile_pool(name="accsA", bufs=1, space="PSUM") as accsA_pool,
                    tc.tile_pool(name="accbA", bufs=1, space="PSUM") as accbA_pool,
                    tc.tile_pool(name="accsB", bufs=1, space="PSUM") as accsB_pool,
                    tc.tile_pool(name="accbB", bufs=1, space="PSUM") as accbB_pool,
                    tc.tile_pool(name="pwp", bufs=1, space="PSUM") as pw_pool,
                ):
                    pools = [(accsA_pool, accbA_pool), (accsB_pool, accbB_pool)]
                    p_all = state.tile([128, TJ * CP], fp8, tag="pall")
                    nc.gpsimd.dma_start(
                        p_all.rearrange("p (r f) -> p r f", r=NCORES)[:],
                        ag_out.rearrange(
                            "(r p f) -> p r f", r=NCORES, p=128
                        )[:],
                    )
                    acc_s0 = accsA_pool.tile([CP, 512], f32, tag="accs0")
                    acc_b0 = accbA_pool.tile([CP, 512], f32, tag="accb0")

                    ks3 = ks_res.rearrange("p (t i) -> p t i", t=TJ)
                    kb3 = kb_res.rearrange("p (t i) -> p t i", t=TJ)

                    def half_accum(p_src, h, acc_s, acc_b, skip_b=False):
                        off = HALVES[h][0]
                        chunks = HALVES[h][3]
                        p3 = p_src.rearrange("p (t c) -> p t c", c=CP)
                        for jp in range(TJ // 2):
                            lhs = p3[:, 2 * jp : 2 * jp + 2, :]
                            first, last = jp == 0, jp == TJ // 2 - 1
                            for co, cl in chunks:
                                nc.tensor.matmul(
                                    acc_s[:, co : co + cl], lhs,
                                    ks3[
                                        :, 2 * jp : 2 * jp + 2,
                                        off + co : off + co + cl,
                                    ],
                                    start=first, stop=last,
                                    perf_mode=mybir.MatmulPerfMode.DoubleRow,
                                )
                                if not skip_b:
                                    nc.tensor.matmul(
                                        acc_b[:, co : co + cl], lhs,
                                        kb3[
                                            :, 2 * jp : 2 * jp + 2,
                                            off + co : off + co + cl,
                                        ],
                                        start=first, stop=last,
                                        perf_mode=mybir.MatmulPerfMode.DoubleRow,
                                    )

                    # ---- iteration 0: A then B ----
                    half_accum(p_all, 0, acc_s0, acc_b0)
                    ag_halves = [half_tail(0, 0, acc_s0, acc_b0)]
                    acc_s1 = accsB_pool.tile([CP, 640], f32, tag="accs1")
                    acc_b1 = accbB_pool.tile([CP, 640], f32, tag="accb1")
                    half_accum(p_all, 1, acc_s1, acc_b1)
                    ag_halves.append(half_tail(0, 1, acc_s1, acc_b1))

                    # ---- iterations 1..4 ----
                    for it in range(1, NITER):
                        p_all = state.tile([128, TJ * CP], fp8, tag="pall")
                        pv = p_all.rearrange(
                            "p (r g c) -> p r g c", r=NCORES, g=TB
                        )
                        for h, (off, ln, nt, _) in enumerate(HALVES):
                            eng = nc.sync if h == 0 else nc.gpsimd
                            eng.dma_start(
                                pv[:, :, (off // 128) : (off // 128) + nt, :],
                                ag_halves[h].rearrange(
                                    "(r p g c) -> p r g c",
                                    r=NCORES, p=128, g=nt,
                                )[:],
                            )
                        ag_halves = []
                        for h, (off, ln, nt, chunks) in enumerate(HALVES):
                            sp, bp = pools[h]
                            acc_s = sp.tile([CP, ln], f32, tag=f"accs{h}")
                            acc_b = bp.tile([CP, ln], f32, tag=f"accb{h}")
                            half_accum(p_all, h, acc_s, acc_b)
                            ag_halves.append(half_tail(it, h, acc_s, acc_b))

    nc.compile()
    return nc


def _host_inputs(unaries, reference_image, spatial_ker_weights,
                 bilateral_ker_weights, compatibility_matrix):
    """Per-core input maps (all host work is O(N*D) layout prep + the
    position-only spatial kernel constant)."""
    u = np.asarray(unaries, np.float32).reshape(N, C)
    img = np.asarray(reference_image, np.float32).reshape(N, 3)
    ws = np.asarray(spatial_ker_weights, np.float32)
    wb = np.asarray(bilateral_ker_weights, np.float32)
    comp = np.asarray(compatibility_matrix, np.float32)

    yy, xx = np.meshgrid(np.arange(H, dtype=np.float32),
                         np.arange(W, dtype=np.float32), indexing="ij")
    pos = np.stack([yy.ravel(), xx.ravel()], axis=1)          # [N, 2]

    # bilateral split: Kb = Gpos * Gcol.  Gcol via fp16 matmul of recentered
    # color features: Xc[j,i] = Fc_j.Fc_i - .5|Fc_j|^2 - .5|Fc_i|^2
    Fc = (img - 0.5) / THETA_BETA                                      # [N, 3]
    sqc = 0.5 * (Fc * Fc).sum(axis=1)                                  # [N]
    ones = np.ones((N, 1), np.float32)
    at = np.concatenate([Fc, ones, -sqc[:, None]], axis=1).T.astype(np.float16)
    bt_full = np.concatenate([Fc, -sqc[:, None], ones], axis=1).T.astype(np.float16)

    # padded u, folded compat
    u_pad = np.full((N, CP), NEG, np.float32)
    u_pad[:, :C] = u
    csf = np.zeros((CP, CP), np.float32)
    cbf = np.zeros((CP, CP), np.float32)
    csf[:C, :C] = ws[:, None] * comp.T          # CsF[c,k] = ws[c] * C[k,c]
    cbf[:C, :C] = wb[:, None] * comp.T

    # spatial kernel: position-only constant
    py, px = pos[:, 0], pos[:, 1]
    in_maps = []
    for r in range(NCORES):
        band = slice(r * BAND, (r + 1) * BAND)
        d2 = (py[:, None] - py[None, band]) ** 2 + (px[:, None] - px[None, band]) ** 2
        kst = np.exp(d2 * (-0.5 / (THETA_GAMMA * THETA_GAMMA))).astype(
            ml_dtypes.float8_e4m3
        )                                                       # [N, BAND]
        gpos = np.exp(d2 * (-0.5 / (THETA_ALPHA * THETA_ALPHA))).astype(
            ml_dtypes.float8_e4m3
        )                                                       # [N, BAND]
        uband = (
            u_pad[band]
            .reshape(TB, 128, CP)
            .transpose(1, 0, 2)
            .reshape(128, TB * CP)
        )
        in_maps.append({
            "atc": at,
            "btc": np.ascontiguousarray(bt_full[:, band]),
            "gpos": gpos,
            "kst": kst,
            "uband": uband,
            "csf": csf,
            "cbf": cbf,
        })
    return in_maps


def _run(in_maps, trace=False, **kw):
    if "nc" not in _CACHE:
        _CACHE["nc"] = _build_nc()
    return run_bass_kernel_spmd(
        _CACHE["nc"], in_maps, list(range(NCORES)), trace=trace, **kw
    )


def _assemble(results):
    bands = []
    for r in range(NCORES):
        arr = results[r]["out"]                              # [128, TB*CP]
        band = arr.reshape(128, TB, CP).transpose(1, 0, 2).reshape(BAND, CP)
        bands.append(band[:, :C])
    return np.concatenate(bands, axis=0).reshape(1, H, W, C).astype(np.float32)


def kernel(unaries, reference_image, spatial_ker_weights,
           bilateral_ker_weights, compatibility_matrix):
    in_maps = _host_inputs(
        unaries, reference_image, spatial_ker_weights,
        bilateral_ker_weights, compatibility_matrix,
    )
    res = _run(in_maps, trace=False)
    return _assemble(res.results)
